# revision 3
# baseline (speedup 1.0000x reference)
"""Trainium2 Bass kernel: MoE top-k router (top-8 of 64 experts + softmax).

Contract: kernel(logits, top_k) takes the FULL inputs (logits [1048576, 64]
f32, top_k == 8) and returns (topk_idx int64 [N, 8], topk_w f32 [N, 8]),
matching jax.lax.top_k + jax.nn.softmax semantics (stable descending order,
ties broken toward the smaller index).

Sharding: data-parallel over tokens across 8 NeuronCores (one SPMD program,
per-core slices fed via run_bass_kernel_spmd). Per core, tokens are laid out
partition-major — partition p owns tokens [p*1024, (p+1)*1024) — so every
DMA moves contiguous multi-KB runs per partition.

Per 128-token group the DVE executes one MAX8 (top-8 values, descending,
exact f32 compare) and one MATCH_VALUE_LOAD+FIND_INDEX8 (stable first-match
indices; the HW match unit skips already-matched positions, so duplicate
values get distinct indices in jax order). These three DVE ops are the
bottleneck (~320 ns per 128 tokens); everything else is kept off the DVE:
exp on ScalarE, softmax-denominator tree-sum and the final scale multiply
on GPSIMD. The reciprocal runs on DVE via the 2-instruction ~2ULP
Newton-Raphson approximation (cheaper than the iterative-divide op).
MAX8s are issued in a phase before the FIND pairs so the DVE streams
back-to-back at its ~126 ns/instruction floor. A small first tile (16
tokens/partition) lets the DVE start before the first full 2 MiB tile lands.
"""

import sys

if "/opt/trn_rl_repo" not in sys.path:
    sys.path.insert(0, "/opt/trn_rl_repo")

import numpy as np

N_TOKENS = 1048576
E = 64             # experts
K = 8              # top-k
NCORES = 8
P = 128            # SBUF partitions
TPC = N_TOKENS // NCORES   # tokens per core = 131072
TPP = TPC // P             # tokens per partition = 1024
T = 64                     # tokens per partition per full tile
RAMP = 16                  # first-tile size (earlier DVE start)

_CACHE = {}


def _build(tpp=TPP, t_tile=T, ramp=RAMP):
    import concourse.bacc as bacc
    import concourse.mybir as mybir
    import concourse.tile as tile

    f32 = mybir.dt.float32
    u16 = mybir.dt.uint16

    n_tok = P * tpp
    if ramp and tpp > 2 * t_tile:
        sizes = [ramp, t_tile - ramp] + [t_tile] * (tpp // t_tile - 1)
    else:
        sizes = [t_tile] * (tpp // t_tile)
    assert sum(sizes) == tpp
    offs = [sum(sizes[:j]) for j in range(len(sizes))]

    nc = bacc.Bacc("TRN2", target_bir_lowering=False, debug=False)
    logits = nc.dram_tensor("logits", [n_tok, E], f32, kind="ExternalInput")
    idx_out = nc.dram_tensor("idx_out", [n_tok, K], u16, kind="ExternalOutput")
    w_out = nc.dram_tensor("w_out", [n_tok, K], f32, kind="ExternalOutput")

    # partition-major: token(p, t) = p*tpp + t
    lg_v = logits.ap().rearrange("(p t) e -> p t e", p=P, t=tpp)
    ix_v = idx_out.ap().rearrange("(p t) k -> p t k", p=P, t=tpp)
    w_v = w_out.ap().rearrange("(p t) k -> p t k", p=P, t=tpp)

    with tile.TileContext(nc) as tc:
        with tc.tile_pool(name="io", bufs=4) as pool:
            for o, tt in zip(offs, sizes):
                x = pool.tile([P, tt, E], f32, tag="x")
                nc.sync.dma_start(x[:], lg_v[:, o:o + tt, :])
                vals = pool.tile([P, tt, K], f32, tag="vals")
                idx = pool.tile([P, tt, K], u16, tag="idx")
                # phase order: all MAX8 first, then the FIND pairs — the DVE
                # then streams each opcode back-to-back without RAW stalls
                for t in range(tt):
                    nc.vector.max(vals[:, t, :], x[:, t, :])
                for t in range(tt):
                    nc.vector.max_index(idx[:, t, :], vals[:, t, :], x[:, t, :])
                ex = pool.tile([P, tt, K], f32, tag="ex")
                nc.scalar.activation(
                    ex[:], vals[:], mybir.ActivationFunctionType.Exp
                )
                # softmax denominator: pairwise tree-sum on GPSIMD (keeps
                # the DVE free for MAX8/FIND_INDEX8, its bottleneck)
                t1 = pool.tile([P, tt, 4], f32, tag="t1")
                t2 = pool.tile([P, tt, 2], f32, tag="t2")
                s = pool.tile([P, tt, 1], f32, tag="s")
                nc.gpsimd.tensor_add(t1[:], ex[:, :, 0:4], ex[:, :, 4:8])
                nc.gpsimd.tensor_add(t2[:], t1[:, :, 0:2], t1[:, :, 2:4])
                nc.gpsimd.tensor_add(s[:], t2[:, :, 0:1], t2[:, :, 1:2])
                r = pool.tile([P, tt, 1], f32, tag="r")
                rs = pool.tile([P, tt, 1], f32, tag="rs")
                nc.vector.reciprocal_approx_accurate(r[:], s[:], rs[:])
                w = pool.tile([P, tt, K], f32, tag="w")
                nc.gpsimd.tensor_mul(w[:], ex[:], r[:].broadcast_to([P, tt, K]))
                nc.sync.dma_start(ix_v[:, o:o + tt, :], idx[:])
                nc.sync.dma_start(w_v[:, o:o + tt, :], w[:])
    nc.compile()
    return nc


def _get_nc():
    if "nc" not in _CACHE:
        _CACHE["nc"] = _build()
    return _CACHE["nc"]


def kernel(logits, top_k):
    logits = np.asarray(logits, dtype=np.float32)
    k = int(np.asarray(top_k))
    assert k == K, f"kernel hardcodes top_k={K}, got {k}"
    assert logits.shape == (N_TOKENS, E), logits.shape

    from concourse.bass_utils import run_bass_kernel_spmd

    nc = _get_nc()
    chunks = logits.reshape(NCORES, TPC, E)
    in_maps = [{"logits": np.ascontiguousarray(chunks[c])} for c in range(NCORES)]
    res = run_bass_kernel_spmd(nc, in_maps, list(range(NCORES)))

    # DRAM row r of each per-core output is token r of that core's slice
    # (the views write token p*1024+t at row p*1024+t), so a plain concat
    # along the token axis reassembles the full outputs.
    idx = np.concatenate([r["idx_out"] for r in res.results], axis=0)
    w = np.concatenate([r["w_out"] for r in res.results], axis=0)
    return idx.astype(np.int64), w.astype(np.float32)


# revision 4
# speedup vs baseline: 1.0050x; 1.0050x over previous
"""Trainium2 Bass kernel: MoE top-k router (top-8 of 64 experts + softmax).

Contract: kernel(logits, top_k) takes the FULL inputs (logits [1048576, 64]
f32, top_k == 8) and returns (topk_idx int64 [N, 8], topk_w f32 [N, 8]),
matching jax.lax.top_k + jax.nn.softmax semantics (stable descending order,
ties broken toward the smaller index).

Sharding: data-parallel over tokens across 8 NeuronCores (one SPMD program,
per-core slices fed via run_bass_kernel_spmd). Per core, tokens are laid out
partition-major — partition p owns tokens [p*1024, (p+1)*1024) — so every
DMA moves contiguous multi-KB runs per partition.

Per 128-token group the DVE executes one MAX8 (top-8 values, descending,
exact f32 compare) and one MATCH_VALUE_LOAD+FIND_INDEX8 (stable first-match
indices; the HW match unit skips already-matched positions, so duplicate
values get distinct indices in jax order). These three DVE ops are the
bottleneck (~320 ns per 128 tokens); everything else is kept off the DVE:
exp on ScalarE, softmax-denominator tree-sum and the final scale multiply
on GPSIMD. The reciprocal runs on DVE via the 2-instruction ~2ULP
Newton-Raphson approximation (cheaper than the iterative-divide op).
MAX8s are issued in a phase before the FIND pairs so the DVE streams
back-to-back at its ~126 ns/instruction floor. A small first tile (16
tokens/partition) lets the DVE start before the first full 2 MiB tile lands.
"""

import sys

if "/opt/trn_rl_repo" not in sys.path:
    sys.path.insert(0, "/opt/trn_rl_repo")

import numpy as np

N_TOKENS = 1048576
E = 64             # experts
K = 8              # top-k
NCORES = 8
P = 128            # SBUF partitions
TPC = N_TOKENS // NCORES   # tokens per core = 131072
TPP = TPC // P             # tokens per partition = 1024
T = 64                     # tokens per partition per full tile
RAMP = 16                  # first-tile size (earlier DVE start)

_CACHE = {}


def _build(tpp=TPP, t_tile=T, ramp=RAMP):
    import concourse.bacc as bacc
    import concourse.mybir as mybir
    import concourse.tile as tile

    f32 = mybir.dt.float32
    u16 = mybir.dt.uint16

    n_tok = P * tpp
    if ramp and tpp > 2 * t_tile:
        sizes = [ramp, t_tile - ramp] + [t_tile] * (tpp // t_tile - 1)
    else:
        sizes = [t_tile] * (tpp // t_tile)
    assert sum(sizes) == tpp
    offs = [sum(sizes[:j]) for j in range(len(sizes))]

    nc = bacc.Bacc("TRN2", target_bir_lowering=False, debug=False)
    logits = nc.dram_tensor("logits", [n_tok, E], f32, kind="ExternalInput")
    idx_out = nc.dram_tensor("idx_out", [n_tok, K], u16, kind="ExternalOutput")
    w_out = nc.dram_tensor("w_out", [n_tok, K], f32, kind="ExternalOutput")

    # partition-major: token(p, t) = p*tpp + t
    lg_v = logits.ap().rearrange("(p t) e -> p t e", p=P, t=tpp)
    ix_v = idx_out.ap().rearrange("(p t) k -> p t k", p=P, t=tpp)
    w_v = w_out.ap().rearrange("(p t) k -> p t k", p=P, t=tpp)

    with tile.TileContext(nc) as tc:
        with tc.tile_pool(name="io", bufs=4) as pool:
            for o, tt in zip(offs, sizes):
                x = pool.tile([P, tt, E], f32, tag="x")
                nc.sync.dma_start(x[:], lg_v[:, o:o + tt, :])
                vals = pool.tile([P, tt, K], f32, tag="vals")
                idx = pool.tile([P, tt, K], u16, tag="idx")
                # phase order: all MAX8 first, then the FIND pairs — the DVE
                # then streams each opcode back-to-back without RAW stalls
                for t in range(tt):
                    nc.vector.max(vals[:, t, :], x[:, t, :])
                for t in range(tt):
                    nc.vector.max_index(idx[:, t, :], vals[:, t, :], x[:, t, :])
                ex = pool.tile([P, tt, K], f32, tag="ex")
                nc.scalar.activation(
                    ex[:], vals[:], mybir.ActivationFunctionType.Exp
                )
                # softmax denominator: pairwise tree-sum on GPSIMD (keeps
                # the DVE free for MAX8/FIND_INDEX8, its bottleneck)
                t1 = pool.tile([P, tt, 4], f32, tag="t1")
                t2 = pool.tile([P, tt, 2], f32, tag="t2")
                s = pool.tile([P, tt, 1], f32, tag="s")
                nc.gpsimd.tensor_add(t1[:], ex[:, :, 0:4], ex[:, :, 4:8])
                nc.gpsimd.tensor_add(t2[:], t1[:, :, 0:2], t1[:, :, 2:4])
                nc.gpsimd.tensor_add(s[:], t2[:, :, 0:1], t2[:, :, 1:2])
                r = pool.tile([P, tt, 1], f32, tag="r")
                rs = pool.tile([P, tt, 1], f32, tag="rs")
                nc.vector.reciprocal_approx_accurate(r[:], s[:], rs[:])
                w = pool.tile([P, tt, K], f32, tag="w")
                nc.gpsimd.tensor_mul(w[:], ex[:], r[:].broadcast_to([P, tt, K]))
                nc.sync.dma_start(ix_v[:, o:o + tt, :], idx[:])
                nc.sync.dma_start(w_v[:, o:o + tt, :], w[:])
    nc.compile()
    return nc


def _get_nc():
    if "nc" not in _CACHE:
        _CACHE["nc"] = _build()
    return _CACHE["nc"]


def kernel(logits, top_k):
    logits = np.asarray(logits, dtype=np.float32)
    k = int(np.asarray(top_k))
    assert k == K, f"kernel hardcodes top_k={K}, got {k}"
    assert logits.shape == (N_TOKENS, E), logits.shape

    from concourse.bass_utils import run_bass_kernel_spmd

    nc = _get_nc()
    chunks = logits.reshape(NCORES, TPC, E)
    in_maps = [{"logits": np.ascontiguousarray(chunks[c])} for c in range(NCORES)]
    # The tunneled devices occasionally fail a run with a transient
    # NRT_EXEC_UNIT_UNRECOVERABLE error; a straight retry recovers.
    last_err = None
    for _attempt in range(3):
        try:
            res = run_bass_kernel_spmd(nc, in_maps, list(range(NCORES)))
            break
        except Exception as e:  # noqa: BLE001 - retry transient device faults
            last_err = e
            import time as _time

            _time.sleep(5.0)
    else:
        raise last_err

    # DRAM row r of each per-core output is token r of that core's slice
    # (the views write token p*1024+t at row p*1024+t), so a plain concat
    # along the token axis reassembles the full outputs.
    idx = np.concatenate([r["idx_out"] for r in res.results], axis=0)
    w = np.concatenate([r["w_out"] for r in res.results], axis=0)
    return idx.astype(np.int64), w.astype(np.float32)
